# revision 54
# baseline (speedup 1.0000x reference)
"""Trainium2 Bass kernel for nn_Decoder1 (linear -> BatchNorm1d -> multistep LIF).

Reference computation (T=4, B=32, N=1024, C=256):
  y[tb,o,n]   = sum_c x[tb,n,c] * W[o,c]                      (TB=128 slices)
  z           = BN(y) over (tb, n) per channel o (training stats, eps=1e-5)
  LIF over t  : v' = (v + z_t)/2 ; s = (v' >= 1) ; v' *= (1-s)
  out[tb,n',c'] = spikes[tb].reshape(C,N).T   (row-major reinterpretation)

Sharding: data-parallel over B (batch) -> 4 b-values x 4 timesteps = 16
(N,C) slices per core. BN statistics via two tiny AllReduces of per-core
(sum, sumsq) — one per slice-half, so the first overlaps phase-1 compute.

Matmul in single-pass fp32r (tf32-like). The ~1e-4 rms error in y flips a
few hundred spikes globally — inside the 2e-2 rel-err budget.

Recompute structure: phase 1 transposes x (PE), rounds it to fp32r into a
persistent SBUF buffer, and runs the matmul once ONLY to feed bn_stats
(y itself is discarded — cheaper than staging 2MB of y through ACT copies).
After the stats AllReduce, phase 2 re-runs the matmul from the stored
fp32r x (PE is otherwise idle there) and feeds BN scale/shift + LIF
directly from PSUM. Identical y both times (deterministic), so BN stats
remain exact for the data actually used.

Layout trick: x rows are loaded in a permuted order (n = 4q+r -> column
j = 256r+128h+q of the transposed moving operand) so the final spike tiles
DMA out to the (TB, C, N)->(TB, N, C) reinterpreted output with contiguous
1KB runs on the DRAM side, and the input DMA gets 4KB contiguous runs.
"""

import numpy as np
from contextlib import ExitStack

import concourse.bass as bass
import concourse.mybir as mybir
import concourse.tile as tile
from concourse.masks import make_identity

F32 = mybir.dt.float32
F32R = mybir.dt.float32r
F16 = mybir.dt.float16
Alu = mybir.AluOpType
ActF = mybir.ActivationFunctionType

N_CORES = 8
T, B, N, C = 4, 32, 1024, 256
B_LOC = B // N_CORES            # 4 batch entries per core
SL = T * B_LOC                  # 16 (N,C) slices per core; sl = bl*4 + t
P = 128
NS_HALF = float(8 * N)          # BN samples per channel per core per slice-half
NS_TOT = float(T * B * N)       # BN samples per channel globally
BN_EPS = 1e-5
SPK = 4.0e9                     # sigmoid step scale: sigmoid(SPK*(v-1)) ~ (v>=1)

SINGLE = False   # test-only: skip the AllReduce (for single-core sim)
AR_SPLIT = True  # two half-batch AllReduces (first overlaps phase-1 compute)
DVE_SPIKE_T = ()   # timesteps whose spike comparison runs on DVE is_ge
GS_SPIKE_T = ()    # timesteps whose spike comparison runs on GpSimd is_ge
NEWTON = 1       # rstd Newton steps
_ctr = [0]


def _legalize_waits(nc, limit=1):
    """This walrus accepts very few semaphore waits per instruction (PE
    matmul: 1).  Hoist excess waits onto same-engine NoOps inserted just
    before the overloaded instruction (same engine => in-order => identical
    semantics)."""
    for f in nc.m.functions:
        for bb in f.blocks:
            new, dirty = [], False
            for ins in bb.instructions:
                si = ins.sync_info
                if si is not None and len(si.on_wait) > limit:
                    waits = list(si.on_wait)
                    for w in waits[:-limit]:
                        _ctr[0] += 1
                        no = mybir.InstNoOp(name=f"zwaitnop-{_ctr[0]}", ins=[], outs=[])
                        no.engine = ins.engine
                        no.sync_info = mybir.SyncInfo(on_wait=[w], on_update=[])
                        new.append(no)
                    ins.sync_info = mybir.SyncInfo(
                        on_wait=waits[-limit:], on_update=list(si.on_update)
                    )
                    dirty = True
                new.append(ins)
            if dirty:
                bb.instructions = new


def _build():
    nc = bass.Bass(num_devices=N_CORES)
    x_in = nc.declare_dram_parameter("x", [SL, N, C], F32, isOutput=False)
    w_in = nc.declare_dram_parameter("W", [C, C], F32, isOutput=False)
    g_in = nc.declare_dram_parameter("gamma", [C], F32, isOutput=False)
    b_in = nc.declare_dram_parameter("beta", [C], F32, isOutput=False)
    out = nc.declare_dram_parameter("out", [SL, N, C], F32, isOutput=True)

    # x rows n = h*512 + q*4 + r loaded so partition=q, free=(h,r,c): the
    # (r,c) block is 4KB-contiguous in DRAM. Transpose chunk (r,h) -> block
    # m = 2r+h, so moving column j = 256r + 128h + q holds row n.
    x_v = x_in.rearrange("s (h q r) c -> s q h r c", h=2, q=128, r=4)
    out_v = out.rearrange("s (r ch cl) q -> s ch cl r q", r=4, ch=2, cl=128)
    w_v = w_in.rearrange("(oh p) c -> p oh c", oh=2, p=128)
    g_v = g_in.rearrange("(oh p) -> p oh", p=128)
    b_v = b_in.rearrange("(oh p) -> p oh", p=128)

    with ExitStack() as ctx:
        tc = ctx.enter_context(tile.TileContext(nc))
        consts = ctx.enter_context(tc.tile_pool(name="consts", bufs=1))
        natp = ctx.enter_context(tc.tile_pool(name="natp", bufs=5))
        xtsp = ctx.enter_context(tc.tile_pool(name="xtsp", bufs=1))
        lifp = ctx.enter_context(tc.tile_pool(name="lifp", bufs=3))
        vpool = ctx.enter_context(tc.tile_pool(name="vpool", bufs=2))
        smallp = ctx.enter_context(tc.tile_pool(name="smallp", bufs=1))

        ident = consts.tile([P, P], F32)
        make_identity(nc, ident)
        ident_r = consts.tile([P, P], F32R)
        nc.vector.tensor_copy(ident_r, ident)

        nspk = consts.tile([P, 1], F32)
        nc.vector.memset(nspk, -SPK)
        pspk = consts.tile([P, 1], F32)
        nc.vector.memset(pspk, SPK)

        # first input slices: issue their DMAs before anything else so the
        # PE pipeline can start; W/gamma/beta follow on the same queue
        nat0 = []
        for sl0 in range(2):
            for h in range(2):
                nat_h = natp.tile([P, 4, C], F32, name="nat", tag="nat")
                nc.sync.dma_start(out=nat_h, in_=x_v[sl0, :, h])
                nat0.append(nat_h)

        # ---- constants: W^T tiles (fp32r), gamma/beta ----
        wnat = consts.tile([P, 2, C], F32, name="wnat")
        nc.sync.dma_start(out=wnat, in_=w_v)
        gam = consts.tile([P, 2], F32)
        nc.sync.dma_start(out=gam, in_=g_v)
        bet = consts.tile([P, 2], F32)
        nc.sync.dma_start(out=bet, in_=b_v)

        wr_nat = smallp.tile([P, 2, C], F32R, name="wr_nat")
        nc.vector.tensor_copy(wr_nat, wnat)
        wt = consts.tile([P, 2, C], F32R, name="wt_r")

        # persistent fp32r transposed x: [sl, ch, j] (replaces a y buffer)
        xts = xtsp.tile([P, SL, 2, 1024], F32R)

        stat6 = smallp.tile([P, 2, 2 * SL, 6], F32, name="stat6")
        ar = []
        sh_d, _ = tc.tile([1, 2, P], F32, space="DRAM", name="sh_d")

        # ---- phase 1: transpose+round x, matmul once for bn_stats ----
        with tc.tile_pool(name="xtps", bufs=2, space="PSUM") as xtps, \
             tc.tile_pool(name="yps", bufs=4, space="PSUM") as yps:

            # wt[:, ch, o] = round_f32r(W[o, ch*128+p])  (stationary tiles)
            wtp = xtps.tile([P, 4, P], F32R, name="wtp", tag="xt_ps")
            for chh in range(2):
                for oh in range(2):
                    nc.tensor.transpose(
                        wtp[:, chh * 2 + oh, :], wr_nat[:, oh, chh * P:(chh + 1) * P],
                        ident_r,
                    )
            for chh in range(2):
                for oh in range(2):
                    nc.scalar.copy(
                        wt[:, chh, oh * P:(oh + 1) * P], wtp[:, chh * 2 + oh, :]
                    )

            def prep(sl):
                if sl < 2:
                    nat = nat0[2 * sl:2 * sl + 2]
                else:
                    nat = []
                    for h in range(2):
                        nat_h = natp.tile([P, 4, C], F32, name="nat", tag="nat")
                        nc.sync.dma_start(out=nat_h, in_=x_v[sl, :, h])
                        nat.append(nat_h)
                for chh in range(2):
                    xt_ps = xtps.tile([P, 1024], F32, name="xt_ps", tag="xt_ps")
                    for r in range(4):
                        for h in range(2):
                            m = 2 * r + h
                            nc.tensor.transpose(
                                xt_ps[:, m * P:(m + 1) * P],
                                nat[h][:, r, chh * P:(chh + 1) * P],
                                ident,
                            )
                    # PSUM->SBUF, rounding to fp32r; parallel across ACT/DVE
                    if chh == 0:
                        nc.vector.tensor_copy(xts[:, sl, chh, :], xt_ps)
                    else:
                        nc.scalar.copy(xts[:, sl, chh, :], xt_ps)

            def stats_mm(sl):
                for oh in range(2):
                    for nsl in range(2):
                        yp = yps.tile([P, 512], F32, name="yp")
                        for chh in range(2):
                            nc.tensor.matmul(
                                yp,
                                wt[:, chh, oh * P:(oh + 1) * P],
                                xts[:, sl, chh, nsl * 512:(nsl + 1) * 512],
                                start=(chh == 0),
                                stop=(chh == 1),
                            )
                        nc.vector.bn_stats(stat6[:, oh, sl * 2 + nsl, :], yp)

            prep(0)
            for sl in range(SL):
                if sl + 1 < SL:
                    prep(sl + 1)
                stats_mm(sl)
                if AR_SPLIT and sl == SL // 2 - 1:
                    ar.append(_stats_math(nc, smallp, stat6, 0))
            ar.append(_stats_math(nc, smallp, stat6, 1 if AR_SPLIT else None))

        gsts = [_stats_comms(nc, tc, smallp, ccs, tag) for ccs, tag in ar]

        # ---- combine halves, BN scale/shift ----
        gst = smallp.tile([P, 4], F32)
        if len(gsts) == 2:
            nc.vector.tensor_tensor(gst, gsts[0], gsts[1], Alu.add)
        else:
            gst = gsts[0]

        mean_g = smallp.tile([P, 2], F32)
        nc.vector.tensor_scalar(mean_g, gst[:, 0:2], 1.0 / NS_TOT, None, Alu.mult)
        u = smallp.tile([P, 2], F32)                    # var + eps
        msq = smallp.tile([P, 2], F32)
        nc.vector.tensor_scalar(u, gst[:, 2:4], 1.0 / NS_TOT, None, Alu.mult)
        nc.vector.tensor_tensor(msq, mean_g, mean_g, Alu.mult)
        nc.vector.tensor_tensor(u, u, msq, Alu.subtract)
        nc.vector.tensor_scalar(u, u, BN_EPS, None, Alu.add)
        # rstd = 1/sqrt(u), Newton-refined (ACT sqrt / DVE recip are approx)
        sq = smallp.tile([P, 2], F32)
        nc.scalar.sqrt(sq, u)
        r = smallp.tile([P, 2], F32)
        nc.vector.reciprocal(r, sq)
        t1 = smallp.tile([P, 2], F32)
        t2 = smallp.tile([P, 2], F32)
        for _ in range(NEWTON):
            nc.vector.tensor_tensor(t1, r, r, Alu.mult)
            nc.vector.tensor_tensor(t2, u, t1, Alu.mult)
            nc.vector.tensor_scalar(t2, t2, -0.5, 1.5, Alu.mult, Alu.add)
            nc.vector.tensor_tensor(r, r, t2, Alu.mult)
        # sc2 = 0.5*gamma*rstd ; sh2 = 0.5*beta - mean*sc2
        sc2 = smallp.tile([P, 2], F32)
        nc.vector.scalar_tensor_tensor(sc2, gam, 0.5, r, Alu.mult, Alu.mult)
        nc.vector.tensor_tensor(t1, mean_g, sc2, Alu.mult)
        sh2 = smallp.tile([P, 2], F32)
        nc.vector.scalar_tensor_tensor(sh2, bet, 0.5, t1, Alu.mult, Alu.subtract)

        # ---- phase 2: recompute matmul from stored fp32r x, BN affine
        # applied per tile (t=0 on DVE from PSUM; t>0 via ACT zh staging),
        # then LIF ----
        with tc.tile_pool(name="yps2", bufs=4, space="PSUM") as yps2:
            for bl in range(B_LOC):
                v = vpool.tile([P, 2, N], F32, name="v")
                for t in range(T):
                    sl = bl * 4 + t
                    ypair = []
                    for oh in range(2):
                        yp2 = yps2.tile([P, 2, 512], F32, name="yp2", tag="yp2")
                        for nsl in range(2):
                            for chh in range(2):
                                nc.tensor.matmul(
                                    yp2[:, nsl, :],
                                    wt[:, chh, oh * P:(oh + 1) * P],
                                    xts[:, sl, chh, nsl * 512:(nsl + 1) * 512],
                                    start=(chh == 0),
                                    stop=(chh == 1),
                                )
                        ypair.append(yp2)
                    yv = [y.rearrange("p a b -> p (a b)") for y in ypair]
                    s = lifp.tile([P, 2, 4, 256], F32, name="s", tag="s")
                    sv = s.rearrange("p o r q -> p o (r q)")
                    if t == 0:
                        # v0 = sc2*y + sh2 straight from PSUM on DVE
                        for oh in range(2):
                            nc.vector.tensor_scalar(
                                v[:, oh, :], yv[oh], sc2[:, oh:oh + 1],
                                sh2[:, oh:oh + 1], Alu.mult, Alu.add,
                            )
                        nc.scalar.activation(sv, v, ActF.Sigmoid, bias=nspk, scale=SPK)
                    else:
                        for oh in range(2):
                            zh = lifp.tile([P, N], F32, name="zh", tag="zh", bufs=2)
                            nc.scalar.activation(
                                zh, yv[oh], ActF.Identity,
                                bias=sh2[:, oh:oh + 1], scale=sc2[:, oh:oh + 1],
                            )
                            nc.vector.scalar_tensor_tensor(
                                v[:, oh, :], v[:, oh, :], 0.5, zh, Alu.mult, Alu.add
                            )
                        nc.scalar.activation(sv, v, ActF.Sigmoid, bias=nspk, scale=SPK)
                    for oh in range(2):
                        nc.sync.dma_start(out=out_v[sl, oh], in_=s[:, oh])
                    if t < 3:
                        v2 = vpool.tile([P, 2, N], F32, name="v")
                        nc.vector.scalar_tensor_tensor(
                            v2, v, 1.0, v, Alu.is_lt, Alu.mult
                        )
                        v = v2

    _legalize_waits(nc)
    return nc


def _stats_math(nc, smallp, stat6, half):
    """Aggregate stat6 for one slice-half into per-core (sum, sumsq) [P, 4].
    Pure DVE math — safe to emit inside the phase-1 PSUM pool scope."""
    if half is None:
        sl_lo, sl_hi, ns = 0, 2 * SL, 2 * NS_HALF
        tag = "f"
    else:
        sl_lo, sl_hi, ns = half * SL, (half + 1) * SL, NS_HALF
        tag = str(half)
    mv = smallp.tile([P, 2, 2], F32, name=f"mv{tag}")
    for oh in range(2):
        nc.vector.bn_aggr(mv[:, oh, :], stat6[:, oh, sl_lo:sl_hi, :])
    ccs = smallp.tile([P, 4], F32, name=f"ccs{tag}")   # [sum0, sum1, ssq0, ssq1]
    msq = smallp.tile([P, 2], F32, name=f"msq{tag}")
    for oh in range(2):
        nc.vector.tensor_scalar(
            ccs[:, oh:oh + 1], mv[:, oh, 0:1], ns, None, Alu.mult
        )
        nc.vector.tensor_tensor(
            msq[:, oh:oh + 1], mv[:, oh, 0:1], mv[:, oh, 0:1], Alu.mult
        )
        nc.vector.scalar_tensor_tensor(
            ccs[:, 2 + oh:3 + oh], mv[:, oh, 1:2], ns, msq[:, oh:oh + 1],
            Alu.bypass, Alu.add,
        )
    # ccs[:, 2+oh] currently = var + mean^2 ; scale to sumsq
    nc.vector.tensor_scalar(ccs[:, 2:4], ccs[:, 2:4], ns, None, Alu.mult)
    return ccs, tag


def _stats_comms(nc, tc, smallp, ccs, tag):
    """DMA the per-core stats out, AllReduce, DMA the global stats back.
    Emitted OUTSIDE the phase-1 pool scope so the pool-release boundary
    (hence phase-2 PSUM allocation) does not wait on the collective."""
    cc_in, _ = tc.tile([P, 4], F32, space="DRAM", name=f"cc_in{tag}")
    cc_out, _ = tc.tile(
        [P, 4], F32, space="DRAM", addr_space="Shared", name=f"cc_out{tag}"
    )
    nc.gpsimd.dma_start(out=cc_in, in_=ccs)
    if not SINGLE:
        nc.gpsimd.collective_compute(
            "AllReduce", Alu.add,
            replica_groups=[list(range(N_CORES))],
            ins=[cc_in[:]], outs=[cc_out[:]],
        )
    gst = smallp.tile([P, 4], F32, name=f"gst{tag}")
    nc.gpsimd.dma_start(out=gst, in_=cc_in if SINGLE else cc_out)
    return gst


_nc_cache = None


def _get_nc():
    global _nc_cache
    if _nc_cache is None:
        _nc_cache = _build()
    return _nc_cache


def _tb_index(core, sl):
    bl, t = sl // T, sl % T
    return t * B + core * B_LOC + bl


def kernel(x, W, gamma, beta, _trace=False, _trace_kwargs=None):
    from concourse.bass_utils import run_bass_kernel_spmd

    x = np.ascontiguousarray(np.asarray(x, dtype=np.float32))
    W = np.ascontiguousarray(np.asarray(W, dtype=np.float32))
    gamma = np.ascontiguousarray(np.asarray(gamma, dtype=np.float32))
    beta = np.ascontiguousarray(np.asarray(beta, dtype=np.float32))

    nc = _get_nc()
    in_maps = []
    for k in range(N_CORES):
        idx = [_tb_index(k, sl) for sl in range(SL)]
        in_maps.append({
            "x": np.ascontiguousarray(x[idx]),
            "W": W, "gamma": gamma, "beta": beta,
        })
    kwargs = dict(_trace_kwargs or {})
    res = run_bass_kernel_spmd(
        nc, in_maps, core_ids=list(range(N_CORES)), trace=_trace, **kwargs
    )
    out = np.empty((T * B, N, C), dtype=np.float32)
    for k in range(N_CORES):
        ok = res.results[k]["out"]
        for sl in range(SL):
            out[_tb_index(k, sl)] = ok[sl]
    if _trace:
        return out, res
    return out


# revision 55
# speedup vs baseline: 1.0505x; 1.0505x over previous
"""Trainium2 Bass kernel for nn_Decoder1 (linear -> BatchNorm1d -> multistep LIF).

Reference computation (T=4, B=32, N=1024, C=256):
  y[tb,o,n]   = sum_c x[tb,n,c] * W[o,c]                      (TB=128 slices)
  z           = BN(y) over (tb, n) per channel o (training stats, eps=1e-5)
  LIF over t  : v' = (v + z_t)/2 ; s = (v' >= 1) ; v' *= (1-s)
  out[tb,n',c'] = spikes[tb].reshape(C,N).T   (row-major reinterpretation)

Sharding: data-parallel over B (batch) -> 4 b-values x 4 timesteps = 16
(N,C) slices per core. BN statistics via two tiny AllReduces of per-core
(sum, sumsq) — one per slice-half, so the first overlaps phase-1 compute.

Matmul in single-pass fp32r (tf32-like). The ~1e-4 rms error in y flips a
few hundred spikes globally — inside the 2e-2 rel-err budget.

Recompute structure: phase 1 transposes x (PE), rounds it to fp32r into a
persistent SBUF buffer, and runs the matmul once ONLY to feed bn_stats
(y itself is discarded — cheaper than staging 2MB of y through ACT copies).
After the stats AllReduce, phase 2 re-runs the matmul from the stored
fp32r x (PE is otherwise idle there) and feeds BN scale/shift + LIF
directly from PSUM. Identical y both times (deterministic), so BN stats
remain exact for the data actually used.

Layout trick: x rows are loaded in a permuted order (n = 4q+r -> column
j = 256r+128h+q of the transposed moving operand) so the final spike tiles
DMA out to the (TB, C, N)->(TB, N, C) reinterpreted output with contiguous
1KB runs on the DRAM side, and the input DMA gets 4KB contiguous runs.
"""

import numpy as np
from contextlib import ExitStack

import concourse.bass as bass
import concourse.mybir as mybir
import concourse.tile as tile
from concourse.masks import make_identity

F32 = mybir.dt.float32
F32R = mybir.dt.float32r
F16 = mybir.dt.float16
Alu = mybir.AluOpType
ActF = mybir.ActivationFunctionType

N_CORES = 8
T, B, N, C = 4, 32, 1024, 256
B_LOC = B // N_CORES            # 4 batch entries per core
SL = T * B_LOC                  # 16 (N,C) slices per core; sl = bl*4 + t
P = 128
NS_HALF = float(8 * N)          # BN samples per channel per core per slice-half
NS_TOT = float(T * B * N)       # BN samples per channel globally
BN_EPS = 1e-5
SPK = 4.0e9                     # sigmoid step scale: sigmoid(SPK*(v-1)) ~ (v>=1)

SINGLE = False   # test-only: skip the AllReduce (for single-core sim)
AR_SPLIT = True  # two half-batch AllReduces (first overlaps phase-1 compute)
DVE_SPIKE_T = ()   # timesteps whose spike comparison runs on DVE is_ge
GS_SPIKE_T = ()    # timesteps whose spike comparison runs on GpSimd is_ge
NEWTON = 1       # rstd Newton steps
_ctr = [0]


def _legalize_waits(nc, limit=1):
    """This walrus accepts very few semaphore waits per instruction (PE
    matmul: 1).  Hoist excess waits onto same-engine NoOps inserted just
    before the overloaded instruction (same engine => in-order => identical
    semantics)."""
    for f in nc.m.functions:
        for bb in f.blocks:
            new, dirty = [], False
            for ins in bb.instructions:
                si = ins.sync_info
                if si is not None and len(si.on_wait) > limit:
                    waits = list(si.on_wait)
                    for w in waits[:-limit]:
                        _ctr[0] += 1
                        no = mybir.InstNoOp(name=f"zwaitnop-{_ctr[0]}", ins=[], outs=[])
                        no.engine = ins.engine
                        no.sync_info = mybir.SyncInfo(on_wait=[w], on_update=[])
                        new.append(no)
                    ins.sync_info = mybir.SyncInfo(
                        on_wait=waits[-limit:], on_update=list(si.on_update)
                    )
                    dirty = True
                new.append(ins)
            if dirty:
                bb.instructions = new


def _build():
    nc = bass.Bass(num_devices=N_CORES)
    x_in = nc.declare_dram_parameter("x", [SL, N, C], F32, isOutput=False)
    w_in = nc.declare_dram_parameter("W", [C, C], F32, isOutput=False)
    g_in = nc.declare_dram_parameter("gamma", [C], F32, isOutput=False)
    b_in = nc.declare_dram_parameter("beta", [C], F32, isOutput=False)
    out = nc.declare_dram_parameter("out", [SL, N, C], F32, isOutput=True)

    # x rows n = h*512 + q*4 + r loaded so partition=q, free=(h,r,c): the
    # (r,c) block is 4KB-contiguous in DRAM. Transpose chunk (r,h) -> block
    # m = 2r+h, so moving column j = 256r + 128h + q holds row n.
    x_v = x_in.rearrange("s (h q r) c -> s q h r c", h=2, q=128, r=4)
    out_v = out.rearrange("s (r ch cl) q -> s ch cl r q", r=4, ch=2, cl=128)
    w_v = w_in.rearrange("(oh p) c -> p oh c", oh=2, p=128)
    g_v = g_in.rearrange("(oh p) -> p oh", p=128)
    b_v = b_in.rearrange("(oh p) -> p oh", p=128)

    with ExitStack() as ctx:
        tc = ctx.enter_context(tile.TileContext(nc))
        consts = ctx.enter_context(tc.tile_pool(name="consts", bufs=1))
        natp = ctx.enter_context(tc.tile_pool(name="natp", bufs=5))
        xtsp = ctx.enter_context(tc.tile_pool(name="xtsp", bufs=1))
        lifp = ctx.enter_context(tc.tile_pool(name="lifp", bufs=3))
        vpool = ctx.enter_context(tc.tile_pool(name="vpool", bufs=2))
        smallp = ctx.enter_context(tc.tile_pool(name="smallp", bufs=1))

        ident = consts.tile([P, P], F32)
        make_identity(nc, ident)
        ident_r = consts.tile([P, P], F32R)
        nc.vector.tensor_copy(ident_r, ident)

        nspk = consts.tile([P, 1], F32)
        nc.vector.memset(nspk, -SPK)
        pspk = consts.tile([P, 1], F32)
        nc.vector.memset(pspk, SPK)

        # first input slices: issue their DMAs before anything else so the
        # PE pipeline can start; W/gamma/beta follow on the same queue
        nat0 = []
        for sl0 in range(2):
            for h in range(2):
                nat_h = natp.tile([P, 4, C], F32, name="nat", tag="nat")
                nc.sync.dma_start(out=nat_h, in_=x_v[sl0, :, h])
                nat0.append(nat_h)

        # ---- constants: W^T tiles (fp32r), gamma/beta ----
        wnat = consts.tile([P, 2, C], F32, name="wnat")
        nc.sync.dma_start(out=wnat, in_=w_v)
        gam = consts.tile([P, 2], F32)
        nc.sync.dma_start(out=gam, in_=g_v)
        bet = consts.tile([P, 2], F32)
        nc.sync.dma_start(out=bet, in_=b_v)

        wr_nat = smallp.tile([P, 2, C], F32R, name="wr_nat")
        nc.vector.tensor_copy(wr_nat, wnat)
        wt = consts.tile([P, 2, C], F32R, name="wt_r")

        # persistent fp32r transposed x: [sl, ch, j] (replaces a y buffer)
        xts = xtsp.tile([P, SL, 2, 1024], F32R)

        stat6 = smallp.tile([P, 2, 2 * SL, 6], F32, name="stat6")
        ar = []
        sh_d, _ = tc.tile([1, 2, P], F32, space="DRAM", name="sh_d")

        # ---- phase 1: transpose+round x, matmul once for bn_stats ----
        with tc.tile_pool(name="xtps", bufs=2, space="PSUM") as xtps, \
             tc.tile_pool(name="yps", bufs=4, space="PSUM") as yps:

            # wt[:, ch, o] = round_f32r(W[o, ch*128+p])  (stationary tiles)
            wtp = xtps.tile([P, 4, P], F32R, name="wtp", tag="xt_ps")
            for chh in range(2):
                for oh in range(2):
                    nc.tensor.transpose(
                        wtp[:, chh * 2 + oh, :], wr_nat[:, oh, chh * P:(chh + 1) * P],
                        ident_r,
                    )
            for chh in range(2):
                for oh in range(2):
                    nc.scalar.copy(
                        wt[:, chh, oh * P:(oh + 1) * P], wtp[:, chh * 2 + oh, :]
                    )

            def prep(sl):
                if sl < 2:
                    nat = nat0[2 * sl:2 * sl + 2]
                else:
                    nat = []
                    for h in range(2):
                        nat_h = natp.tile([P, 4, C], F32, name="nat", tag="nat")
                        nc.sync.dma_start(out=nat_h, in_=x_v[sl, :, h])
                        nat.append(nat_h)
                for chh in range(2):
                    xt_ps = xtps.tile([P, 1024], F32, name="xt_ps", tag="xt_ps")
                    for r in range(4):
                        for h in range(2):
                            m = 2 * r + h
                            nc.tensor.transpose(
                                xt_ps[:, m * P:(m + 1) * P],
                                nat[h][:, r, chh * P:(chh + 1) * P],
                                ident,
                            )
                    # PSUM->SBUF, rounding to fp32r; parallel across ACT/DVE
                    if chh == 0:
                        nc.vector.tensor_copy(xts[:, sl, chh, :], xt_ps)
                    else:
                        nc.scalar.copy(xts[:, sl, chh, :], xt_ps)

            def stats_mm(sl):
                for oh in range(2):
                    for nsl in range(2):
                        yp = yps.tile([P, 512], F32, name="yp")
                        for chh in range(2):
                            nc.tensor.matmul(
                                yp,
                                wt[:, chh, oh * P:(oh + 1) * P],
                                xts[:, sl, chh, nsl * 512:(nsl + 1) * 512],
                                start=(chh == 0),
                                stop=(chh == 1),
                            )
                        nc.vector.bn_stats(stat6[:, oh, sl * 2 + nsl, :], yp)

            prep(0)
            for sl in range(SL):
                if sl + 1 < SL:
                    prep(sl + 1)
                stats_mm(sl)
                if AR_SPLIT and sl == SL // 2 - 1:
                    ar.append(_stats_math(nc, smallp, stat6, 0))
            ar.append(_stats_math(nc, smallp, stat6, 1 if AR_SPLIT else None))

        gsts = [_stats_comms(nc, tc, smallp, ccs, tag) for ccs, tag in ar]

        # ---- combine halves, BN scale/shift ----
        gst = smallp.tile([P, 4], F32)
        if len(gsts) == 2:
            nc.vector.tensor_tensor(gst, gsts[0], gsts[1], Alu.add)
        else:
            gst = gsts[0]

        mean_g = smallp.tile([P, 2], F32)
        nc.vector.tensor_scalar(mean_g, gst[:, 0:2], 1.0 / NS_TOT, None, Alu.mult)
        u = smallp.tile([P, 2], F32)                    # var + eps
        msq = smallp.tile([P, 2], F32)
        nc.vector.tensor_scalar(u, gst[:, 2:4], 1.0 / NS_TOT, None, Alu.mult)
        nc.vector.tensor_tensor(msq, mean_g, mean_g, Alu.mult)
        nc.vector.tensor_tensor(u, u, msq, Alu.subtract)
        nc.vector.tensor_scalar(u, u, BN_EPS, None, Alu.add)
        # rstd = 1/sqrt(u), Newton-refined (ACT sqrt / DVE recip are approx)
        sq = smallp.tile([P, 2], F32)
        nc.scalar.sqrt(sq, u)
        r = smallp.tile([P, 2], F32)
        nc.vector.reciprocal(r, sq)
        t1 = smallp.tile([P, 2], F32)
        t2 = smallp.tile([P, 2], F32)
        for _ in range(NEWTON):
            nc.vector.tensor_tensor(t1, r, r, Alu.mult)
            nc.vector.tensor_tensor(t2, u, t1, Alu.mult)
            nc.vector.tensor_scalar(t2, t2, -0.5, 1.5, Alu.mult, Alu.add)
            nc.vector.tensor_tensor(r, r, t2, Alu.mult)
        # sc2 = 0.5*gamma*rstd ; sh2 = 0.5*beta - mean*sc2
        sc2 = smallp.tile([P, 2], F32)
        nc.vector.scalar_tensor_tensor(sc2, gam, 0.5, r, Alu.mult, Alu.mult)
        nc.vector.tensor_tensor(t1, mean_g, sc2, Alu.mult)
        sh2 = smallp.tile([P, 2], F32)
        nc.vector.scalar_tensor_tensor(sh2, bet, 0.5, t1, Alu.mult, Alu.subtract)

        # ---- phase 2: recompute matmul from stored fp32r x, BN affine
        # applied per tile (t=0 on DVE from PSUM; t>0 via ACT zh staging),
        # then LIF ----
        with tc.tile_pool(name="yps2", bufs=4, space="PSUM") as yps2:
            for bl in range(B_LOC):
                v = vpool.tile([P, 2, N], F32, name="v")
                for t in range(T):
                    sl = bl * 4 + t
                    ypair = []
                    for oh in range(2):
                        yp2 = yps2.tile([P, 2, 512], F32, name="yp2", tag="yp2")
                        for nsl in range(2):
                            for chh in range(2):
                                nc.tensor.matmul(
                                    yp2[:, nsl, :],
                                    wt[:, chh, oh * P:(oh + 1) * P],
                                    xts[:, sl, chh, nsl * 512:(nsl + 1) * 512],
                                    start=(chh == 0),
                                    stop=(chh == 1),
                                )
                        ypair.append(yp2)
                    yv = [y.rearrange("p a b -> p (a b)") for y in ypair]
                    s = lifp.tile([P, 2, 4, 256], F32, name="s", tag="s")
                    sv = s.rearrange("p o r q -> p o (r q)")
                    if t == 0:
                        # v0 = sc2*y + sh2 straight from PSUM on DVE
                        for oh in range(2):
                            nc.vector.tensor_scalar(
                                v[:, oh, :], yv[oh], sc2[:, oh:oh + 1],
                                sh2[:, oh:oh + 1], Alu.mult, Alu.add,
                            )
                        nc.scalar.activation(sv, v, ActF.Sigmoid, bias=nspk, scale=SPK)
                    else:
                        for oh in range(2):
                            zh = lifp.tile([P, N], F32, name="zh", tag="zh", bufs=2)
                            nc.scalar.activation(
                                zh, yv[oh], ActF.Identity,
                                bias=sh2[:, oh:oh + 1], scale=sc2[:, oh:oh + 1],
                            )
                            nc.vector.scalar_tensor_tensor(
                                v[:, oh, :], v[:, oh, :], 0.5, zh, Alu.mult, Alu.add
                            )
                        nc.scalar.activation(sv, v, ActF.Sigmoid, bias=nspk, scale=SPK)
                    for oh in range(2):
                        nc.sync.dma_start(out=out_v[sl, oh], in_=s[:, oh])
                    if t < 3:
                        v2 = vpool.tile([P, 2, N], F32, name="v")
                        nc.vector.scalar_tensor_tensor(
                            v2, v, 1.0, v, Alu.is_lt, Alu.mult
                        )
                        v = v2

    _legalize_waits(nc)
    return nc


def _stats_math(nc, smallp, stat6, half):
    """Aggregate stat6 for one slice-half into per-core (sum, sumsq) [P, 4].
    Pure DVE math — safe to emit inside the phase-1 PSUM pool scope."""
    if half is None:
        sl_lo, sl_hi, ns = 0, 2 * SL, 2 * NS_HALF
        tag = "f"
    else:
        sl_lo, sl_hi, ns = half * SL, (half + 1) * SL, NS_HALF
        tag = str(half)
    mv = smallp.tile([P, 2, 2], F32, name=f"mv{tag}")
    for oh in range(2):
        nc.vector.bn_aggr(mv[:, oh, :], stat6[:, oh, sl_lo:sl_hi, :])
    ccs = smallp.tile([P, 4], F32, name=f"ccs{tag}")   # [sum0, sum1, ssq0, ssq1]
    msq = smallp.tile([P, 2], F32, name=f"msq{tag}")
    for oh in range(2):
        nc.vector.tensor_scalar(
            ccs[:, oh:oh + 1], mv[:, oh, 0:1], ns, None, Alu.mult
        )
        nc.vector.tensor_tensor(
            msq[:, oh:oh + 1], mv[:, oh, 0:1], mv[:, oh, 0:1], Alu.mult
        )
        nc.vector.scalar_tensor_tensor(
            ccs[:, 2 + oh:3 + oh], mv[:, oh, 1:2], ns, msq[:, oh:oh + 1],
            Alu.bypass, Alu.add,
        )
    # ccs[:, 2+oh] currently = var + mean^2 ; scale to sumsq
    nc.vector.tensor_scalar(ccs[:, 2:4], ccs[:, 2:4], ns, None, Alu.mult)
    return ccs, tag


def _stats_comms(nc, tc, smallp, ccs, tag):
    """DMA the per-core stats out, AllReduce, DMA the global stats back.
    Emitted OUTSIDE the phase-1 pool scope so the pool-release boundary
    (hence phase-2 PSUM allocation) does not wait on the collective."""
    cc_in, _ = tc.tile([P, 4], F32, space="DRAM", name=f"cc_in{tag}")
    cc_out, _ = tc.tile(
        [P, 4], F32, space="DRAM", addr_space="Shared", name=f"cc_out{tag}"
    )
    nc.sync.dma_start(out=cc_in, in_=ccs)
    if not SINGLE:
        nc.gpsimd.collective_compute(
            "AllReduce", Alu.add,
            replica_groups=[list(range(N_CORES))],
            ins=[cc_in[:]], outs=[cc_out[:]],
        )
    gst = smallp.tile([P, 4], F32, name=f"gst{tag}")
    nc.sync.dma_start(out=gst, in_=cc_in if SINGLE else cc_out)
    return gst


_nc_cache = None


def _get_nc():
    global _nc_cache
    if _nc_cache is None:
        _nc_cache = _build()
    return _nc_cache


def _tb_index(core, sl):
    bl, t = sl // T, sl % T
    return t * B + core * B_LOC + bl


def kernel(x, W, gamma, beta, _trace=False, _trace_kwargs=None):
    from concourse.bass_utils import run_bass_kernel_spmd

    x = np.ascontiguousarray(np.asarray(x, dtype=np.float32))
    W = np.ascontiguousarray(np.asarray(W, dtype=np.float32))
    gamma = np.ascontiguousarray(np.asarray(gamma, dtype=np.float32))
    beta = np.ascontiguousarray(np.asarray(beta, dtype=np.float32))

    nc = _get_nc()
    in_maps = []
    for k in range(N_CORES):
        idx = [_tb_index(k, sl) for sl in range(SL)]
        in_maps.append({
            "x": np.ascontiguousarray(x[idx]),
            "W": W, "gamma": gamma, "beta": beta,
        })
    kwargs = dict(_trace_kwargs or {})
    res = run_bass_kernel_spmd(
        nc, in_maps, core_ids=list(range(N_CORES)), trace=_trace, **kwargs
    )
    out = np.empty((T * B, N, C), dtype=np.float32)
    for k in range(N_CORES):
        ok = res.results[k]["out"]
        for sl in range(SL):
            out[_tb_index(k, sl)] = ok[sl]
    if _trace:
        return out, res
    return out
